# revision 12
# baseline (speedup 1.0000x reference)
"""Trainium2 Bass kernel for nn_MoE_32332513804634.

MoE: 16 routed experts (top-6, softmax-then-bias routing) + dense shared
expert, T=4096 tokens, D=2048, H=1408, HS=2816, fp32.

Strategy (8 NeuronCores, SPMD):
  - Host computes the gate (cheap: 0.27 GFLOP) and per-expert token lists.
  - Expert parallelism with load-balanced segmentation: expert token lists
    are carved into pieces and packed into uniform per-core "segments"
    (1 big slot of cap 2048 + k small slots of cap 512 per core), so every
    core executes an identical instruction stream over 3584 token slots.
  - Each segment runs SwiGLU for its expert over its gathered tokens with
    float32r matmuls (full PE rate, ~1e-4 rel precision), the per-token
    combine weight applied as a per-partition scale on the PSUM->SBUF copy.
  - b2 is folded into an augmented row of W2^T via a constant ones-row in
    the lhsT (so out = c * (h @ W2^T + b2) in one accumulation chain).
  - Shared expert is tensor-parallel over its 2816 hidden dim (352 rows
    per core, padded to 384), same pipeline, bs2 added on core 0 only.
  - Host scatters segment outputs back to token rows and sums partials.
"""

import sys
import numpy as np

sys.path.insert(0, "/opt/trn_rl_repo")

import concourse.bass as bass  # noqa: E402
import concourse.tile as tile  # noqa: E402
from concourse import bacc, mybir  # noqa: E402
from concourse.bass_utils import run_bass_kernel_spmd  # noqa: E402

T = 4096
D = 2048
H = 1408
E = 16
TOP_K = 6
HS = 2816
N_CORES = 8
HM = H // 128          # 11
KO = D // 128          # 16
HS_PAD = 384           # shared hidden shard (352) padded to 3*128
HMS = HS_PAD // 128    # 3
BIG_CAP = 2048
SMALL_CAP = 512
F32 = mybir.dt.float32
F32R = mybir.dt.float32r
BF16 = mybir.dt.bfloat16

# matmul operand dtype: bf16 halves HBM traffic (the kernel is DMA-bound in
# f32r) at ~2.5e-3 relative error; accumulation stays fp32 in PSUM.
MM_DT = BF16

_PROGRAM_CACHE: dict = {}


def _to_mm(a):
    if MM_DT == BF16:
        import ml_dtypes
        return np.ascontiguousarray(a).astype(ml_dtypes.bfloat16)
    return np.ascontiguousarray(a)


def _host_gate(xf, gate_w, gate_b):
    """Numpy replica of the reference gate. Returns cw [T, E] dense combine
    weights and per-expert token lists (ascending)."""
    scores = xf @ gate_w.T
    m = scores.max(axis=-1, keepdims=True)
    p = np.exp(scores - m, dtype=np.float32)
    probs = p / p.sum(axis=-1, keepdims=True)
    biased = probs + gate_b
    idx = np.argpartition(biased, E - TOP_K, axis=1)[:, E - TOP_K:]
    mask = np.zeros((xf.shape[0], E), dtype=bool)
    mask[np.arange(xf.shape[0])[:, None], idx] = True
    cw = np.where(mask, probs, 0.0).astype(np.float32)
    toks = [np.flatnonzero(mask[:, e]).astype(np.int64) for e in range(E)]
    return cw, toks


def _plan_segments(counts):
    """Carve expert token counts into pieces and pack into per-core slots.

    Returns (seg_caps, assignment) where seg_caps is the per-core slot
    capacity tuple and assignment[core][slot] = list of (expert, start, n)
    -- here each slot holds exactly one piece (expert, start offset into
    that expert's token list, piece length) or None for an empty slot.
    """
    order = np.argsort(counts)[::-1]
    bigs = []      # (expert, start, n) with n <= BIG_CAP
    smalls = []    # (expert, start, n) with n <= SMALL_CAP
    rema = []      # remainders to chop into smalls
    for i, e in enumerate(order):
        c = int(counts[e])
        if i < N_CORES:
            n = min(c, BIG_CAP)
            bigs.append((int(e), 0, n))
            if c > n:
                rema.append((int(e), n, c - n))
        else:
            rema.append((int(e), 0, c))
    for e, s0, rem in rema:
        o = 0
        while o < rem:
            n = min(SMALL_CAP, rem - o)
            smalls.append((e, s0 + o, n))
            o += n
    n_small_slots = -(-len(smalls) // N_CORES)  # ceil
    seg_caps = (BIG_CAP,) + (SMALL_CAP,) * n_small_slots
    assignment = []
    for c in range(N_CORES):
        slots = [bigs[c]]
        for s in range(n_small_slots):
            k = s * N_CORES + c
            slots.append(smalls[k] if k < len(smalls) else None)
        assignment.append(slots)
    return seg_caps, assignment


def _build_program(seg_caps):
    """Build the SPMD Bass program for the given per-core slot capacities."""
    nc = bacc.Bacc("TRN2", debug=False, num_devices=N_CORES)

    ins = {}
    outs = {}

    def din(name, shape, dt=MM_DT):
        ins[name] = nc.dram_tensor(name, list(shape), dt, kind="ExternalInput").ap()
        return ins[name]

    def dout(name, shape, dt=F32):
        outs[name] = nc.dram_tensor(name, list(shape), dt, kind="ExternalOutput").ap()
        return outs[name]

    for s, cap in enumerate(seg_caps):
        din(f"xg{s}", (D, cap))
        din(f"w1t{s}", (D, H))
        din(f"w3t{s}", (D, H))
        din(f"w2ta{s}", (H + 128, D))
        din(f"b1_{s}", (128, HM), F32)
        din(f"b3_{s}", (128, HM), F32)
        din(f"scl{s}", (128, cap // 128), F32)
        dout(f"oe{s}", (cap, D))
    din("xt", (D, T))
    din("ws1s", (D, HS_PAD))
    din("ws3s", (D, HS_PAD))
    din("ws2sa", (HS_PAD + 128, D))
    din("bs1", (128, HMS), F32)
    din("bs3", (128, HMS), F32)
    din("onesrow", (128, 512))
    dout("zs", (T, D))

    with tile.TileContext(nc) as tc:
        with (
            tc.tile_pool(name="xpool", bufs=2) as xpool,
            tc.tile_pool(name="hpool", bufs=2) as hpool,
            tc.tile_pool(name="wcol", bufs=4) as wcol,
            tc.tile_pool(name="w2pool", bufs=2) as w2pool,
            tc.tile_pool(name="tmp", bufs=2) as tmp,
            tc.tile_pool(name="opool", bufs=2) as opool,
            tc.tile_pool(name="cpool", bufs=1) as cpool,
            tc.tile_pool(name="pp", bufs=2, space="PSUM") as pp,
        ):
            # constant ones-row tile used as the augmented lhsT k-subtile
            ones = cpool.tile([128, 512], MM_DT, tag="ones")
            nc.sync.dma_start(ones[:], ins["onesrow"][:])

            def mlp_segment(xg_ap, w1_ap, w3_ap, w2_ap, b1_ap, b3_ap,
                            scl_ap, out_ap, cap, n_hm, tag, scale_one):
                """One expert segment: out = scale * (swiglu(x) @ W2a^T)."""
                n_k2 = n_hm + 1
                x3 = xg_ap.rearrange("(ko p) t -> p ko t", p=128)
                w1c3 = w1_ap.rearrange("(ko p) h -> p ko h", p=128)
                w3c3 = w3_ap.rearrange("(ko p) h -> p ko h", p=128)
                w23 = w2_ap.rearrange("(k p) d -> p k d", p=128)

                b1sb = cpool.tile([128, n_hm], F32, tag=f"b1{tag}")
                b3sb = cpool.tile([128, n_hm], F32, tag=f"b3{tag}")
                nc.sync.dma_start(b1sb[:], b1_ap)
                nc.sync.dma_start(b3sb[:], b3_ap)
                if not scale_one:
                    sclsb = cpool.tile([128, cap // 128], F32, tag=f"scl{tag}")
                    nc.sync.dma_start(sclsb[:], scl_ap)

                n_tc = cap // 512
                for t in range(n_tc):
                    xsb = xpool.tile([128, KO, 512], MM_DT, tag="xg")
                    nc.sync.dma_start(xsb[:], x3[:, :, t * 512:(t + 1) * 512])
                    hsb = hpool.tile([128, HM, 512], MM_DT, tag="h")
                    for hm in range(n_hm):
                        w1t_ = wcol.tile([128, KO, 128], MM_DT, tag="w1c")
                        nc.sync.dma_start(w1t_[:], w1c3[:, :, hm * 128:(hm + 1) * 128])
                        w3t_ = wcol.tile([128, KO, 128], MM_DT, tag="w3c")
                        nc.sync.dma_start(w3t_[:], w3c3[:, :, hm * 128:(hm + 1) * 128])
                        ps1 = pp.tile([128, 512], F32, tag="ph1")
                        for ko in range(KO):
                            nc.tensor.matmul(ps1[:], w1t_[:, ko, :], xsb[:, ko, :],
                                             start=(ko == 0), stop=(ko == KO - 1))
                        ps3 = pp.tile([128, 512], F32, tag="ph3")
                        for ko in range(KO):
                            nc.tensor.matmul(ps3[:], w3t_[:, ko, :], xsb[:, ko, :],
                                             start=(ko == 0), stop=(ko == KO - 1))
                        h1t = tmp.tile([128, 512], F32, tag="h1t")
                        nc.scalar.activation(h1t[:], ps1[:],
                                             mybir.ActivationFunctionType.Silu,
                                             bias=b1sb[:, hm:hm + 1])
                        h3t = tmp.tile([128, 512], F32, tag="h3t")
                        nc.scalar.activation(h3t[:], ps3[:],
                                             mybir.ActivationFunctionType.Identity,
                                             bias=b3sb[:, hm:hm + 1])
                        nc.vector.tensor_mul(hsb[:, hm, :], h1t[:], h3t[:])
                    # second matmul: out rows = tokens
                    for dm in range(4):
                        w2sb = w2pool.tile([128, n_k2, 512], MM_DT, tag="w2s")
                        nc.sync.dma_start(
                            w2sb[:], w23[:, :, dm * 512:(dm + 1) * 512])
                        for tch in range(4):
                            tok0 = t * 512 + tch * 128
                            ps2 = pp.tile([128, 512], F32, tag="po")
                            for k in range(n_k2):
                                if k < n_hm:
                                    lhsT = hsb[:, k, tch * 128:(tch + 1) * 128]
                                else:
                                    lhsT = ones[:, tch * 128:(tch + 1) * 128]
                                nc.tensor.matmul(ps2[:], lhsT,
                                                 w2sb[:, k, :],
                                                 start=(k == 0), stop=(k == n_k2 - 1))
                            osb = opool.tile([128, 512], F32, tag="osb")
                            if scale_one:
                                nc.vector.tensor_copy(osb[:], ps2[:])
                            else:
                                col = tok0 // 128
                                nc.vector.tensor_scalar_mul(
                                    osb[:], ps2[:], sclsb[:, col:col + 1])
                            nc.sync.dma_start(
                                out_ap[tok0:tok0 + 128, dm * 512:(dm + 1) * 512],
                                osb[:])

            for s, cap in enumerate(seg_caps):
                mlp_segment(ins[f"xg{s}"], ins[f"w1t{s}"], ins[f"w3t{s}"],
                            ins[f"w2ta{s}"], ins[f"b1_{s}"], ins[f"b3_{s}"],
                            ins[f"scl{s}"], outs[f"oe{s}"], cap, HM,
                            f"e{s}", False)
            # shared expert (hidden-sharded, all tokens, no combine scale)
            mlp_segment(ins["xt"], ins["ws1s"], ins["ws3s"], ins["ws2sa"],
                        ins["bs1"], ins["bs3"], None, outs["zs"], T, HMS,
                        "sh", True)

    nc.compile()
    return nc


def kernel(x, gate_w, gate_b, w1, b1, w2, b2, w3, b3,
           ws1, bs1, ws2, bs2, ws3, bs3):
    x = np.asarray(x, np.float32)
    xf = np.ascontiguousarray(x.reshape(-1, D))
    gate_w = np.asarray(gate_w, np.float32)
    gate_b = np.asarray(gate_b, np.float32)
    w1 = np.asarray(w1, np.float32)
    b1 = np.asarray(b1, np.float32)
    w2 = np.asarray(w2, np.float32)
    b2 = np.asarray(b2, np.float32)
    w3 = np.asarray(w3, np.float32)
    b3 = np.asarray(b3, np.float32)
    ws1 = np.asarray(ws1, np.float32)
    bs1 = np.asarray(bs1, np.float32)
    ws2 = np.asarray(ws2, np.float32)
    bs2 = np.asarray(bs2, np.float32)
    ws3 = np.asarray(ws3, np.float32)
    bs3 = np.asarray(bs3, np.float32)

    cw, toks = _host_gate(xf, gate_w, gate_b)
    counts = np.array([len(t) for t in toks])
    seg_caps, assignment = _plan_segments(counts)

    if seg_caps not in _PROGRAM_CACHE:
        _PROGRAM_CACHE[seg_caps] = _build_program(seg_caps)
    nc = _PROGRAM_CACHE[seg_caps]

    xT = np.ascontiguousarray(xf.T)  # [D, T]
    xT_mm = _to_mm(xT)

    # per-expert transposed weights (computed once, shared across pieces)
    w1t = {}
    w3t = {}
    w2ta = {}
    need = sorted({p[0] for slots in assignment for p in slots if p is not None})
    for e in need:
        w1t[e] = _to_mm(w1[e].T)
        w3t[e] = _to_mm(w3[e].T)
        a = np.zeros((H + 128, D), np.float32)
        a[:H] = w2[e].T
        a[H] = b2[e]
        w2ta[e] = _to_mm(a)

    # shared expert shards
    hs_per = HS // N_CORES  # 352

    in_maps = []
    for c in range(N_CORES):
        m = {}
        for s, cap in enumerate(seg_caps):
            piece = assignment[c][s]
            xg = np.zeros((D, cap), np.float32)
            scl = np.zeros(cap, np.float32)
            if piece is None:
                e = need[0]
                m[f"w1t{s}"] = w1t[e]
                m[f"w3t{s}"] = w3t[e]
                m[f"w2ta{s}"] = w2ta[e]
                m[f"b1_{s}"] = np.zeros((128, HM), np.float32)
                m[f"b3_{s}"] = np.zeros((128, HM), np.float32)
            else:
                e, s0, n = piece
                tk = toks[e][s0:s0 + n]
                xg[:, :n] = xT[:, tk]
                scl[:n] = cw[tk, e]
                m[f"w1t{s}"] = w1t[e]
                m[f"w3t{s}"] = w3t[e]
                m[f"w2ta{s}"] = w2ta[e]
                m[f"b1_{s}"] = np.ascontiguousarray(
                    b1[e].reshape(HM, 128).T)
                m[f"b3_{s}"] = np.ascontiguousarray(
                    b3[e].reshape(HM, 128).T)
            m[f"xg{s}"] = _to_mm(xg)
            m[f"scl{s}"] = np.ascontiguousarray(
                scl.reshape(cap // 128, 128).T)
        # shared shard
        r0 = c * hs_per
        ws1p = np.zeros((D, HS_PAD), np.float32)
        ws1p[:, :hs_per] = ws1[r0:r0 + hs_per].T
        ws3p = np.zeros((D, HS_PAD), np.float32)
        ws3p[:, :hs_per] = ws3[r0:r0 + hs_per].T
        ws2a = np.zeros((HS_PAD + 128, D), np.float32)
        ws2a[:hs_per] = ws2[:, r0:r0 + hs_per].T
        if c == 0:
            ws2a[HS_PAD] = bs2
        bs1p = np.zeros(HS_PAD, np.float32)
        bs1p[:hs_per] = bs1[r0:r0 + hs_per]
        bs3p = np.zeros(HS_PAD, np.float32)
        bs3p[:hs_per] = bs3[r0:r0 + hs_per]
        m["xt"] = xT_mm
        m["ws1s"] = _to_mm(ws1p)
        m["ws3s"] = _to_mm(ws3p)
        m["ws2sa"] = _to_mm(ws2a)
        m["bs1"] = np.ascontiguousarray(bs1p.reshape(HMS, 128).T)
        m["bs3"] = np.ascontiguousarray(bs3p.reshape(HMS, 128).T)
        onesrow = np.zeros((128, 512), np.float32)
        onesrow[0] = 1.0
        m["onesrow"] = _to_mm(onesrow)
        in_maps.append(m)

    res = run_bass_kernel_spmd(nc, in_maps, list(range(N_CORES)))

    # host combine: scatter segment outputs + sum shared partials
    y = np.zeros((T, D), np.float32)
    for c in range(N_CORES):
        for s, cap in enumerate(seg_caps):
            piece = assignment[c][s]
            if piece is None:
                continue
            e, s0, n = piece
            tk = toks[e][s0:s0 + n]
            y[tk] += res.results[c][f"oe{s}"][:n]
        y += res.results[c]["zs"]
    return y.reshape(x.shape).astype(np.float32)
